# revision 44
# baseline (speedup 1.0000x reference)
"""AdaptiveConv Trainium2 kernel — 8-core SPMD, batch-sharded, 1-D Winograd.

Per full batch:
  x [16, 256, 64, 64] f32, w [16, 512] f32,
  filter_bank [8, 256, 256, 3, 3], dense_fw (512->8), dense_mod (512->256).
  fbw = softmax(w @ Wfw + bfw)                  [16, 8]
  filters = einsum('bfchw,nb->nfchw', bank, fbw)
  filters *= (w @ Wmod + bmod + 1)[n, f]
  norm[n,kh,kw] = sqrt(max(sum_{f,c} filters^2, 1e-8)); filters /= norm
  out[n] = conv2d_same(x[n], filters[n])        [16, 256, 64, 64]

Sharding: batch N=16 over 8 cores (2 samples each); params + the filter
bank replicated, bank loaded ONCE per core and kept SBUF-resident.

The conv runs as F(2,3) 1-D Winograd along W (direct over kh):
  per output pair (2t, 2t+1), with o[t]=x[w=2t-1], e[t]=x[w=2t]:
    xt0 = o[t]-o[t+1]   xt1 = e[t]+o[t+1]
    xt2 = o[t+1]-e[t]   xt3 = e[t]-e[t+1]
    ft0 = g0, ft1 = g0+g1+g2, ft2 = g0-g1+g2, ft3 = g2   (per kh, demodded)
    y_p = sum_{kh,c} ft_p * xt_p  (4 PSUM accumulations, K=768 each)
    even = 0.5*(y1+y2) + y0 ; odd = 0.5*(y1-y2) - y3
  => 24 matmuls per 16-row chunk instead of 36: 2/3 the PE work.

Schedule: the bank DMA (9.4 MB) gates everything (the demod norm needs
the full mix).  DMA rings process descriptors in order and share
bandwidth fairly per ring, so the bank goes first on the sync ring with
sample-1's x right behind it (never competing), sample-0's x races on
the gpsimd ring (the ct0-half of the bank window has PE slack anyway),
and the small params ride the scalar ring with dense biases applied via
K=1 matmuls instead of [P,*] broadcast DMAs.  The PE mixes chase the
naturally staggered bank-tile arrivals in two PSUM-budgeted phases
((0,0)+(1,0)-chunks-0-2 on ct0 tiles, then (0,1)+(1,0)-chunks-3-4 on
ct1), with modulate drains + square-accumulates folded per 512-col
chunk.  The conv-start critical chain is minimized: per-tap unit-stride
demod split across DVE/ACT with the demod-only taps (p=0,3) first,
per-kh unit-stride G-adds split DVE/Pool, chunk 1 runs p-order
(0,3,1,2), and an early dummy Sqrt keeps the ACT table resident.
Sample-1's ct1 mix runs on the PE as three small-PSUM passes between
sample-0 conv chunks; its x-transform and filter chain run as ~0.7us
DVE work items consumed between chunk drains.
"""

import os
import sys

import numpy as np

if "/opt/trn_rl_repo" not in sys.path:
    sys.path.insert(0, "/opt/trn_rl_repo")

import concourse.bacc as bacc_mod
import concourse.mybir as mybir
import concourse.tile as tile
from concourse.bass_utils import run_bass_kernel_spmd

N_CORES = 8
NS = 2            # samples per core
C = 256
F = 256
H = W = 64
KK = 3
TAPS = KK * KK    # 9
NF = 8
WD = 512
P = 128
CT = C // P       # 2 c tiles
FT = F // P       # 2 f tiles
KO = WD // P      # 4 contraction tiles for the dense layers
HP = H + 2        # 66 rows in the winograd input (h = -1..64)
NT = W // 2       # 32 output pairs per row
EO = NT + 2       # 34 cols per deinterleaved odd/even plane
HCH = 16          # output rows per conv chunk
CHN = H // HCH    # 4 chunks per (sample, f-tile)
CHL = HCH * NT    # 512 positions per chunk
MIX_CHUNKS = [(0, 512), (512, 512), (1024, 512), (1536, 512), (2048, 256)]
# taps covered by each mix chunk (tap t = cols [256t, 256t+256))
CHUNK_TAPS = [(0, 1), (2, 3), (4, 5), (6, 7), (8,)]
EPS = 1e-8

USE_BF16 = os.environ.get("KERNEL_F32", "") != "1"
WARM_MM = int(os.environ.get("KERNEL_WARM_MM", "2"))

LAST = None       # BassKernelResults of the most recent run (for test.py)


def _build():
    f32 = mybir.dt.float32
    cdt = mybir.dt.bfloat16 if USE_BF16 else f32

    nc = bacc_mod.Bacc()
    xdi_d = nc.declare_dram_parameter("xdi", [NS, CT, P, 2, HP, EO], cdt,
                                      isOutput=False)
    w_d = nc.declare_dram_parameter("wv_t", [P, KO, NS], f32, isOutput=False)
    bank_d = nc.declare_dram_parameter("bank_t", [NF, C, TAPS, F], cdt,
                                       isOutput=False)
    fww_d = nc.declare_dram_parameter("fw_w", [P, KO, NF], cdt, isOutput=False)
    fwb_d = nc.declare_dram_parameter("fw_b", [NF], cdt, isOutput=False)
    mdw_d = nc.declare_dram_parameter("md_w", [P, KO, F], cdt, isOutput=False)
    mdb_d = nc.declare_dram_parameter("md_b", [F], cdt, isOutput=False)
    ident_d = nc.declare_dram_parameter("ident", [P, P], cdt, isOutput=False)
    out_d = nc.declare_dram_parameter("out", [NS, F, H, W], f32, isOutput=True)

    mm = mybir.AluOpType.mult
    aa = mybir.AluOpType.add
    ss = mybir.AluOpType.subtract
    ACT = mybir.ActivationFunctionType

    with tile.TileContext(nc) as tc, \
         tc.tile_pool(name="const", bufs=1) as const_p, \
         tc.tile_pool(name="small", bufs=2) as small_p, \
         tc.tile_pool(name="bcast", bufs=2) as bc_p, \
         tc.tile_pool(name="diag", bufs=16) as diag_p, \
         tc.tile_pool(name="bank", bufs=16) as bk_p, \
         tc.tile_pool(name="xdi", bufs=2) as xdi_p, \
         tc.tile_pool(name="xt", bufs=8) as xt_p, \
         tc.tile_pool(name="acc", bufs=3) as acc_p, \
         tc.tile_pool(name="tmp", bufs=2) as tmp_p, \
         tc.tile_pool(name="filt", bufs=2) as filt_p, \
         tc.tile_pool(name="f12", bufs=2) as f12_p, \
         tc.tile_pool(name="scr", bufs=2) as scr_p, \
         tc.tile_pool(name="fs", bufs=2) as fs_p, \
         tc.tile_pool(name="outs", bufs=2) as out_p, \
         tc.tile_pool(name="ps", bufs=8, space="PSUM") as ps_p:

        # preload ACT tables (Exp/Square/Sqrt) with a self-contained tile
        tblw = small_p.tile([P, 2], f32, name="tblw")
        nc.vector.memset(tblw, 0.5)
        nc.scalar.activation(tblw, tblw, ACT.Exp)
        nc.scalar.activation(tblw, tblw, ACT.Square)
        nc.scalar.activation(tblw, tblw, ACT.Sqrt)

        # ---- bank DMA first (the long pole) on the sync ring; the ring
        # processes descriptors in order, so xq1 enqueued right after the
        # bank never steals bank bandwidth.
        bkr = [[None] * NF for _ in range(CT)]
        for ct in range(CT):
            for b in range(NF):
                bk = bk_p.tile([P, TAPS, F], cdt, tag="bk", name=f"bk{ct}_{b}")
                # one DMA per mix chunk (tap-pair): overlap tracking
                # releases each individual mix matmul as soon as its own
                # columns land, so the PE chases the bank at ~120KB
                # granularity instead of per-590KB-tile
                cs = ct * P
                for t0, t1 in ((0, 2), (2, 4), (4, 6), (6, 8), (8, 9)):
                    nc.sync.dma_start(bk[:, t0:t1, :],
                                      bank_d[b, cs:cs + P, t0:t1, :])
                bkr[ct][b] = bk

        # ---- x sample 0 (gpsimd ring, races the bank) -------------------
        xq0 = []
        for ct in range(CT):
            xq = xdi_p.tile([P, 2, HP, EO], cdt, tag="xdi", name=f"xq0_{ct}")
            nc.gpsimd.dma_start(xq, xdi_d[0, ct, :, :, :, :])
            xq0.append(xq)

        # ---- x sample 1, behind the bank on the same sync ring: the ring
        # serves it only after the bank, and its descriptor wait (xdi buf
        # reuse, freed by the xt0 transforms) resolves before the ring
        # reaches it.
        xq1 = []
        for ct in range(CT):
            xq = xdi_p.tile([P, 2, HP, EO], cdt, tag="xdi", name=f"xq1_{ct}")
            nc.sync.dma_start(xq, xdi_d[1, ct, :, :, :, :])
            xq1.append(xq)

        # ---- small parameter loads (scalar ring) ------------------------
        # biases as single-partition rows, added via a K=1 matmul — the
        # [P,*] broadcast DMAs they replace were ring-hogs.
        wt = const_p.tile([P, KO, NS], f32)
        nc.scalar.dma_start(wt, w_d[:, :, :])
        fww = const_p.tile([P, KO, NF], cdt)
        nc.scalar.dma_start(fww, fww_d[:, :, :])
        fwb1 = const_p.tile([1, NF], cdt)
        nc.scalar.dma_start(fwb1, fwb_d[:][None, :])
        ident = const_p.tile([P, P], cdt)
        nc.scalar.dma_start(ident, ident_d[:, :])
        mdb1 = const_p.tile([1, F], cdt)
        nc.scalar.dma_start(mdb1, mdb_d[:][None, :])
        # mdw (256 KB, 85% of param bytes) rides the gpsimd ring BEHIND
        # xq0: its consumer chain (dense head -> w1rep -> mix modulate
        # drains) first needs it ~5us after it lands, and the scalar ring
        # finishing early gives the bank a bigger early bandwidth share.
        mdw = const_p.tile([P, KO, F], cdt)
        nc.gpsimd.dma_start(mdw, mdw_d[:, :, :])

        ones_row = const_p.tile([1, P], cdt)
        nc.vector.memset(ones_row, 1.0)

        # ---- dense heads, replicated across all 128 partitions ----------
        fbw_bc = []
        w1rep = []
        for n in range(NS):
            wrep = bc_p.tile([P, KO, P], cdt, tag="wrep")
            nc.vector.tensor_copy(
                wrep, wt[:, :, n][:, :, None].to_broadcast((P, KO, P)))
            ps_l = ps_p.tile([P, 512], f32, tag="ps", name=f"psl{n}")
            for ko in range(KO):
                nc.tensor.matmul(ps_l[:, :NF], lhsT=wrep[:, ko, :],
                                 rhs=fww[:, ko, :], start=(ko == 0),
                                 stop=False)
            nc.tensor.matmul(ps_l[:, :NF], lhsT=ones_row, rhs=fwb1,
                             start=False, stop=True)
            # softmax denominator cancels through the per-tap demod norm
            fb = bc_p.tile([P, NF], f32, tag="fbw_bc")
            nc.scalar.activation(fb, ps_l[:, :NF], ACT.Exp)
            fbw_bc.append(fb)

            ps_sc = ps_p.tile([P, 512], f32, tag="ps", name=f"pssc{n}")
            for ko in range(KO):
                nc.tensor.matmul(ps_sc[:, :F], lhsT=wrep[:, ko, :],
                                 rhs=mdw[:, ko, :], start=(ko == 0),
                                 stop=False)
            nc.tensor.matmul(ps_sc[:, :F], lhsT=ones_row, rhs=mdb1,
                             start=False, stop=True)
            w1 = bc_p.tile([P, F], cdt, tag="w1_bc")
            nc.scalar.activation(w1, ps_sc[:, :F], ACT.Copy, bias=1.0)
            w1r = bc_p.tile([P, TAPS, F], cdt, tag="w1rep")
            nc.vector.tensor_copy(
                w1r, w1[:, None, :].to_broadcast((P, TAPS, F)))
            w1rep.append(w1r)

        # warm psum: fresh short-lived tile (never held across pool phases)
        ps_wa = ps_p.tile([P, 512], f32, tag="ps", name="warm_a")
        ones_sb = const_p.tile([P, P], cdt)
        nc.vector.memset(ones_sb, 1.0)

        # diag(fbw_b) weight tiles for the PE mixes (samples 0 and 1)
        fbwI = [[], []]
        for n in range(NS):
            for b in range(NF):
                dg = diag_p.tile([P, P], cdt, tag="diag", name=f"dg{n}_{b}")
                nc.vector.tensor_scalar_mul(dg, ident, fbw_bc[n][:, b:b + 1])
                fbwI[n].append(dg)
                if WARM_MM and n == 0:
                    for _ in range(2):
                        nc.tensor.matmul(ps_wa[:, :P], lhsT=dg, rhs=dg,
                                         start=True, stop=True)

        # ---- x-tilde transform for sample 0 (DVE, unit-stride bf16) -----
        # xq[ct] planes: [:,0]=odd (w=2t-1), [:,1]=even (w=2t)
        def xt_transform(xq, pool, tag):
            xts = []
            for ct in range(CT):
                o = xq[ct][:, 0, :, :]
                e = xq[ct][:, 1, :, :]
                pl = []
                for p in range(4):
                    t = pool.tile([P, HP, NT], cdt, tag=tag,
                                  name=f"xt{tag}{ct}_{p}")
                    pl.append(t)
                nc.vector.tensor_tensor(pl[0], o[:, :, 0:NT], o[:, :, 1:NT + 1], ss)
                nc.vector.tensor_tensor(pl[1], e[:, :, 0:NT], o[:, :, 1:NT + 1], aa)
                nc.vector.tensor_tensor(pl[2], o[:, :, 1:NT + 1], e[:, :, 0:NT], ss)
                nc.vector.tensor_tensor(pl[3], e[:, :, 0:NT], e[:, :, 1:NT + 1], ss)
                xts.append(pl)
            return xts

        xt0 = xt_transform(xq0, xt_p, "xt")

        # ---- mix bookkeeping --------------------------------------------
        acc = [[None] * CT for _ in range(NS)]
        qt = [[None] * CT for _ in range(NS)]

        def mix_alloc(n, ct):
            a = acc_p.tile([P, TAPS, F], cdt, tag="acc", name=f"acc{n}_{ct}")
            scr = tmp_p.tile([P, TAPS, F], cdt, tag="tmp", name=f"sq{n}_{ct}")
            q = small_p.tile([P, TAPS], f32, tag="q", bufs=4, name=f"q{n}_{ct}")
            acc[n][ct] = a
            qt[n][ct] = q
            return a, scr, q

        a00, scr00, q00 = mix_alloc(0, 0)
        a10, scr10, q10 = mix_alloc(1, 0)
        a01, scr01, q01 = mix_alloc(0, 1)

        def mix_psums(n, ct, cis):
            return {ci: ps_p.tile([P, 512], f32, tag="ps",
                                  name=f"mix{n}{ct}_{ci}")
                    for ci in cis}

        def mix_mms(n, ct, pss, cis, b):
            bf = bkr[ct][b].rearrange("p t f -> p (t f)")
            for ci in cis:
                off, csz = MIX_CHUNKS[ci]
                nc.tensor.matmul(pss[ci][:, :csz], lhsT=fbwI[n][b],
                                 rhs=bf[:, off:off + csz],
                                 start=(b == 0), stop=(b == NF - 1))

        def mix_drain(n, ct, pss, cis, scr, q, eng):
            a = acc[n][ct]
            af = a.rearrange("p t f -> p (t f)")
            w1f = w1rep[n].rearrange("p t f -> p (t f)")
            for ci in cis:
                off, csz = MIX_CHUNKS[ci]
                eng.tensor_tensor(af[:, off:off + csz], pss[ci][:, :csz],
                                  w1f[:, off:off + csz], mm)
                for tp in CHUNK_TAPS[ci]:
                    nc.scalar.activation(scr[:, tp, :], a[:, tp, :],
                                         ACT.Square, accum_out=q[:, tp:tp + 1])

        # PHASE 1 (ct0 tiles): (0,0) full + (1,0) chunks 0-2; 8 psums live.
        ps00 = mix_psums(0, 0, range(5))
        ps10a = mix_psums(1, 0, range(3))
        for b in range(NF):
            mix_mms(0, 0, ps00, range(5), b)
            mix_mms(1, 0, ps10a, range(3), b)
        # drains must run on DVE/ACT (Pool cannot read PSUM)
        mix_drain(0, 0, ps00, range(5), scr00, q00, nc.vector)
        nc.scalar.activation(tblw, tblw, ACT.Sqrt)   # keep Sqrt table hot
        mix_drain(1, 0, ps10a, range(3), scr10, q10, nc.vector)

        # PHASE 2 (ct1 tiles): (0,1) full + (1,0) chunks 3-4; 7 psums live.
        ps01 = mix_psums(0, 1, range(5))
        ps10b = mix_psums(1, 0, (3, 4))
        for b in range(NF):
            mix_mms(0, 1, ps01, range(5), b)
            mix_mms(1, 0, ps10b, (3, 4), b)
        mix_drain(0, 1, ps01, range(5), scr01, q01, nc.vector)

        # ---- norm + demod + winograd filter transform for sample 0 ------
        def norm_chain(n):
            qs = small_p.tile([P, TAPS], cdt, tag="qs", name=f"qs{n}")
            nc.vector.tensor_tensor(qs, qt[n][0], qt[n][1], aa)
            ps_nrm = ps_p.tile([P, 512], f32, tag="ps", name=f"psn{n}")
            nc.tensor.matmul(ps_nrm[:, :TAPS], lhsT=ones_sb, rhs=qs,
                             start=True, stop=True)
            nall = small_p.tile([P, TAPS], f32, tag="nall", name=f"na{n}")
            nc.vector.tensor_scalar_max(nall, ps_nrm[:, :TAPS], EPS)
            sq = small_p.tile([P, TAPS], f32, tag="sq", name=f"sv{n}")
            nc.scalar.activation(sq, nall, ACT.Sqrt)
            ninv = small_p.tile([P, TAPS], f32, tag="ninv", name=f"ni{n}")
            nc.vector.reciprocal(ninv, sq)
            return ninv

        ninv0 = norm_chain(0)

        # filt holds the demodded taps (ft0 = tap kw0, ft3 = tap kw2);
        # f12 holds ft1 = g0+g1+g2 and ft2 = g0-g1+g2 per kh.  Demod runs
        # per-tap (unit-stride, per-partition 1/norm scalar) split across
        # DVE and ACT, taps for the demod-only conv planes (p=0,3) first so
        # chunk 1 can start; the G-adds run per-kh unit-stride, ct0 on DVE
        # and ct1 on Pool.
        DEMOD_ORDER = (0, 3, 6, 2, 5, 8, 1, 4, 7)

        def filter_transform(n, ninv, fl_tiles, fx_tiles):
            for i, tp in enumerate(DEMOD_ORDER):
                for ct in range(CT):
                    src = acc[n][ct][:, tp, :]
                    dst = fl_tiles[ct][:, tp, :]
                    if (2 * i + ct) % 2 == 0:
                        nc.vector.tensor_scalar_mul(dst, src,
                                                    ninv[:, tp:tp + 1])
                    else:
                        nc.scalar.activation(dst, src, ACT.Copy,
                                             scale=ninv[:, tp:tp + 1])
            for ct in range(CT):
                fl, fx = fl_tiles[ct], fx_tiles[ct]
                eng = nc.vector if ct == 0 else nc.gpsimd
                s3 = fs_p.tile([P, KK, F], cdt, tag="fs", name=f"fs{n}_{ct}")
                for kh in range(KK):
                    eng.tensor_tensor(s3[:, kh, :], fl[:, kh * KK, :],
                                      fl[:, kh * KK + 2, :], aa)
                    eng.tensor_tensor(fx[:, 0, kh, :], s3[:, kh, :],
                                      fl[:, kh * KK + 1, :], aa)
                    eng.tensor_tensor(fx[:, 1, kh, :], s3[:, kh, :],
                                      fl[:, kh * KK + 1, :], ss)

        filt0 = [filt_p.tile([P, TAPS, F], cdt, tag="filt", name=f"fl0_{ct}")
                 for ct in range(CT)]
        f120 = [f12_p.tile([P, 2, KK, F], cdt, tag="f12", name=f"fx0_{ct}")
                for ct in range(CT)]
        filter_transform(0, ninv0, filt0, f120)

        # sample-1 ct0 tail drains AFTER the s0 filter chain: they overlap
        # conv chunk 1 instead of sitting on the conv-start critical path.
        mix_drain(1, 0, ps10b, (3, 4), scr10, q10, nc.vector)

        acc11 = bk_p.tile([P, TAPS, F], cdt, tag="bk", name="acc1_1")

        # ---- conv + drains, interleaved with sample-1 prep --------------
        def conv_sample(n, filt, f12, xts, vwork, per_chunk, post_mm=None):
            """vwork: list of callables issuing ~0.8us of engine work each,
            consumed per_chunk at a time between chunk drains.  post_mm:
            {chunk_idx: fn} PE work inserted right after that chunk's MMs."""
            wi = 0
            first = True
            ch = 0
            for ft in range(FT):
                fsl = slice(ft * P, (ft + 1) * P)
                for hc in range(CHN):
                    pss = [ps_p.tile([P, CHL], f32, tag="ps",
                                     name=f"cv{n}{ft}{hc}_{p}")
                           for p in range(4)]
                    # first chunk: run the demod-only taps (p=0,3) first so
                    # the PE can start before the f12 G-adds are done.
                    p_order = (0, 3, 1, 2) if first else (1, 2, 0, 3)
                    first = False
                    for p in p_order:
                        k = 0
                        for kh in range(KK):
                            for ct in range(CT):
                                if p == 0:
                                    lhs = filt[ct][:, kh * KK, fsl]
                                elif p == 3:
                                    lhs = filt[ct][:, kh * KK + 2, fsl]
                                else:
                                    lhs = f12[ct][:, p - 1, kh, fsl]
                                xf = xts[ct][p].rearrange("p h t -> p (h t)")
                                off = (hc * HCH + kh) * NT
                                nc.tensor.matmul(
                                    pss[p][:, :], lhsT=lhs,
                                    rhs=xf[:, off:off + CHL],
                                    start=(k == 0), stop=(k == 2 * KK - 1))
                                k += 1
                    if post_mm and ch in post_mm:
                        post_mm[ch]()
                    ch += 1
                    osb = out_p.tile([P, HCH, W], f32, tag="osb",
                                     name=f"osb{n}{ft}{hc}")
                    ov = osb.rearrange("p h (t two) -> p h t two", two=2)
                    a1 = scr_p.tile([P, HCH, NT], f32, tag="a1",
                                    name=f"a1_{n}{ft}{hc}")
                    a2 = scr_p.tile([P, HCH, NT], f32, tag="a2",
                                    name=f"a2_{n}{ft}{hc}")
                    dv = scr_p.tile([P, HCH, NT], f32, tag="d",
                                    name=f"d{n}{ft}{hc}")
                    p1 = pss[1].rearrange("p (h t) -> p h t", t=NT)
                    p2 = pss[2].rearrange("p (h t) -> p h t", t=NT)
                    p0 = pss[0].rearrange("p (h t) -> p h t", t=NT)
                    p3 = pss[3].rearrange("p (h t) -> p h t", t=NT)
                    # one-PSUM-input-per-op drain, spread ACT/Pool/DVE:
                    #   a1 = 0.5*y1 ; a2 = 0.5*y2 (ACT, psum->sbuf)
                    #   d = a1 - a2 (Pool) ; a1 <- a1 + a2 (DVE)
                    #   even = (a1+a2) + y0 ; odd = -y3 + d (DVE)
                    # the very last chunk drains in row-halves so the final
                    # DMA (and the epilogue behind it) starts sooner
                    last = (n == 1 and ft == FT - 1 and hc == CHN - 1)
                    for r0, r1 in ([(0, 8), (8, HCH)] if last
                                   else [(0, HCH)]):
                        rs = slice(r0, r1)
                        nc.scalar.activation(a1[:, rs], p1[:, rs],
                                             ACT.Copy, scale=0.5)
                        nc.scalar.activation(a2[:, rs], p2[:, rs],
                                             ACT.Copy, scale=0.5)
                        nc.gpsimd.tensor_tensor(dv[:, rs], a1[:, rs],
                                                a2[:, rs], ss)
                        nc.vector.tensor_tensor(a1[:, rs], a1[:, rs],
                                                a2[:, rs], aa)
                        nc.vector.tensor_tensor(ov[:, rs, :, 0], a1[:, rs],
                                                p0[:, rs], aa)
                        nc.vector.scalar_tensor_tensor(ov[:, rs, :, 1],
                                                       p3[:, rs], -1.0,
                                                       dv[:, rs], mm, aa)
                        nc.sync.dma_start(
                            out_d[n, fsl, hc * HCH + r0:hc * HCH + r1, :],
                            osb[:, rs])
                    for _ in range(per_chunk):
                        if wi < len(vwork):
                            vwork[wi]()
                            wi += 1
            while wi < len(vwork):
                vwork[wi]()
                wi += 1

        # ---- sample-1 ct1 mix on the PE, interleaved between s0 conv
        # chunks in two PSUM-budgeted passes (p1 = chunks 0-2, p2 = 3-4).
        scr11 = tmp_p.tile([P, TAPS, F], cdt, tag="tmp", name="sq1_1")
        q11 = small_p.tile([P, TAPS], f32, tag="q", bufs=4, name="q1_1")
        acc[1][1] = acc11
        qt[1][1] = q11

        def mk_mix11_pass(cis):
            # one full 8-bank accumulation over <=2 psum chunks + drain;
            # inserted between s0 conv chunks (live psums: conv 4 + 2)
            def fn():
                pss = mix_psums(1, 1, cis)
                # high_priority: schedule the pass as one tight PE block —
                # left alone the scheduler interleaves these 1:1 with conv
                # matmuls, doubling the time until the drain can run and
                # head-of-line-blocking the DVE queue behind it
                with tc.high_priority(offset=60):
                    for b in range(NF):
                        mix_mms(1, 1, pss, cis, b)
                mix_drain(1, 1, pss, cis, scr11, q11, nc.vector)
            return fn

        mix11_passes = {1: mk_mix11_pass((0, 1)), 3: mk_mix11_pass((2, 3)),
                        5: mk_mix11_pass((4,))}

        filt1 = [bk_p.tile([P, TAPS, F], cdt, tag="bk", name=f"fl1_{ct}")
                 for ct in range(CT)]
        f121 = [bk_p.tile([P, 2, KK, F], cdt, tag="bk", name=f"fx1_{ct}")
                for ct in range(CT)]

        def mk_norm_filt1():
            def fn():
                ninv1 = norm_chain(1)
                filter_transform(1, ninv1, filt1, f121)
            return fn

        def mk_xt1(ct, p, half, eng):
            # half-plane slices of the sample-1 x-tilde transform
            r0, r1 = (0, HP // 2 + 1) if half == 0 else (HP // 2 + 1, HP)
            def fn():
                o = xq1[ct][:, 0, r0:r1, :]
                e = xq1[ct][:, 1, r0:r1, :]
                t = bk_p.tile([P, HP, NT], cdt, tag="bk", name=f"xt1{ct}_{p}") \
                    if half == 0 else xt1[ct][p]
                d = t[:, r0:r1, :]
                if p == 0:
                    eng.tensor_tensor(d, o[:, :, 0:NT], o[:, :, 1:NT + 1], ss)
                elif p == 1:
                    eng.tensor_tensor(d, e[:, :, 0:NT], o[:, :, 1:NT + 1], aa)
                elif p == 2:
                    eng.tensor_tensor(d, o[:, :, 1:NT + 1], e[:, :, 0:NT], ss)
                else:
                    eng.tensor_tensor(d, e[:, :, 0:NT], e[:, :, 1:NT + 1], ss)
                if half == 0:
                    xt1[ct][p] = t
            return fn

        xt1 = [[None] * 4 for _ in range(CT)]
        xt1_items = [mk_xt1(ct, p, half, nc.vector)
                     for ct in range(CT) for p in range(4)
                     for half in range(2)]
        # norm1 placed so its PE matmul fires ~1 chunk after the last
        # mix(1,1) squares land (q11 ready ~chunk 5.5)
        vwork = xt1_items[:15] + [mk_norm_filt1()] + xt1_items[15:]

        conv_sample(0, filt0, f120, xt0, vwork, per_chunk=3,
                    post_mm=mix11_passes)
        conv_sample(1, filt1, f121, xt1, [], per_chunk=0)

    nc.compile()
    return nc


def kernel(x, w, filter_bank, dense_fw_w, dense_fw_b, dense_mod_w, dense_mod_b):
    global LAST
    x = np.ascontiguousarray(np.asarray(x, dtype=np.float32))
    w = np.ascontiguousarray(np.asarray(w, dtype=np.float32))
    xdt = np.float32
    if USE_BF16:
        import ml_dtypes
        xdt = ml_dtypes.bfloat16
    NB = x.shape[0]
    # deinterleaved, padded odd/even planes:
    #   odd[t]  = x[w=2t-1] (t=0..33, zeros at w=-1 and w=65)
    #   even[t] = x[w=2t]   (t=0..33, zeros at w=64.. )
    # rows r=0..65 map to h=r-1 with zero padding at h=-1, 64.
    xr = x.reshape(NB, CT, P, H, W)
    xdi_all = np.zeros((NB, CT, P, 2, HP, EO), dtype=xdt)
    xdi_all[:, :, :, 0, 1:H + 1, 1:NT + 1] = xr[:, :, :, :, 1::2]
    xdi_all[:, :, :, 1, 1:H + 1, 0:NT] = xr[:, :, :, :, 0::2]
    fb = np.asarray(filter_bank, dtype=np.float32)
    # [b, f, c, kh, kw] -> [b, c, (kh kw), f]
    bank_t = np.ascontiguousarray(
        np.transpose(fb, (0, 2, 3, 4, 1)).reshape(NF, C, TAPS, F))
    if USE_BF16:
        import ml_dtypes
        bank_t = bank_t.astype(ml_dtypes.bfloat16)

    trace = os.environ.get("KERNEL_TRACE", "") == "1"
    if trace:
        import types

        import concourse.bass_utils as bu
        bu.upload_artifacts = lambda tmpdir: tmpdir
        if "antenv.axon_hooks" not in sys.modules:
            from trn_agent_boot.trn_boot import _ntff_profile_via_ctypes
            hook = _ntff_profile_via_ctypes("/opt/axon/libaxon_pjrt.so")
            mod = types.ModuleType("antenv.axon_hooks")
            mod.get_axon_ntff_profile_hook = lambda: hook
            sys.modules["antenv.axon_hooks"] = mod

    # dense weights pre-transposed to [P, KO, *] so the DMA is contiguous
    fww_t = np.ascontiguousarray(
        np.asarray(dense_fw_w, np.float32).reshape(KO, P, NF)
        .transpose(1, 0, 2).astype(xdt))
    mdw_t = np.ascontiguousarray(
        np.asarray(dense_mod_w, np.float32).reshape(KO, P, F)
        .transpose(1, 0, 2).astype(xdt))

    ncores = int(os.environ.get("KERNEL_NCORES", N_CORES))
    nc = _build()
    in_maps = []
    for core in range(ncores):
        sl = slice(core * NS, (core + 1) * NS)
        w_t = np.ascontiguousarray(
            w[sl].reshape(NS, KO, P).transpose(2, 1, 0))
        ident = np.eye(P, dtype=xdt)
        in_maps.append({
            "ident": ident,
            "xdi": np.ascontiguousarray(xdi_all[sl]),
            "wv_t": w_t,
            "bank_t": bank_t,
            "fw_w": fww_t,
            "fw_b": np.ascontiguousarray(
                np.asarray(dense_fw_b, np.float32).astype(xdt)),
            "md_w": mdw_t,
            "md_b": np.ascontiguousarray(
                np.asarray(dense_mod_b, np.float32).astype(xdt)),
        })
    kwargs = {}
    if trace:
        import tempfile
        base = os.environ.get("KERNEL_TRACE_DIR", "/tmp/ktrace")
        os.makedirs(base, exist_ok=True)
        tdir = tempfile.mkdtemp(dir=base)
        print(f"trace dir: {tdir}", flush=True)
        kwargs = dict(trace=True, tmpdir=tdir)
    LAST = run_bass_kernel_spmd(nc, in_maps, core_ids=list(range(ncores)),
                                **kwargs)
    return np.concatenate([LAST.results[i]["out"] for i in range(ncores)],
                          axis=0)


# revision 46
# speedup vs baseline: 1.0918x; 1.0918x over previous
"""AdaptiveConv Trainium2 kernel — 8-core SPMD, batch-sharded, 1-D Winograd.

Per full batch:
  x [16, 256, 64, 64] f32, w [16, 512] f32,
  filter_bank [8, 256, 256, 3, 3], dense_fw (512->8), dense_mod (512->256).
  fbw = softmax(w @ Wfw + bfw)                  [16, 8]
  filters = einsum('bfchw,nb->nfchw', bank, fbw)
  filters *= (w @ Wmod + bmod + 1)[n, f]
  norm[n,kh,kw] = sqrt(max(sum_{f,c} filters^2, 1e-8)); filters /= norm
  out[n] = conv2d_same(x[n], filters[n])        [16, 256, 64, 64]

Sharding: batch N=16 over 8 cores (2 samples each); params + the filter
bank replicated, bank loaded ONCE per core and kept SBUF-resident.

The conv runs as F(2,3) 1-D Winograd along W (direct over kh):
  per output pair (2t, 2t+1), with o[t]=x[w=2t-1], e[t]=x[w=2t]:
    xt0 = o[t]-o[t+1]   xt1 = e[t]+o[t+1]
    xt2 = o[t+1]-e[t]   xt3 = e[t]-e[t+1]
    ft0 = g0, ft1 = g0+g1+g2, ft2 = g0-g1+g2, ft3 = g2   (per kh, demodded)
    y_p = sum_{kh,c} ft_p * xt_p  (4 PSUM accumulations, K=768 each)
    even = 0.5*(y1+y2) + y0 ; odd = 0.5*(y1-y2) - y3
  => 24 matmuls per 16-row chunk instead of 36: 2/3 the PE work.

Schedule: the bank DMA (9.4 MB) gates everything (the demod norm needs
the full mix).  DMA rings process descriptors in order and share
bandwidth fairly per ring, so the bank goes first on the sync ring with
sample-1's x right behind it (never competing), sample-0's x races on
the gpsimd ring (the ct0-half of the bank window has PE slack anyway),
and the small params ride the scalar ring with dense biases applied via
K=1 matmuls instead of [P,*] broadcast DMAs.  The PE mixes chase the
naturally staggered bank-tile arrivals in two PSUM-budgeted phases
((0,0)+(1,0)-chunks-0-2 on ct0 tiles, then (0,1)+(1,0)-chunks-3-4 on
ct1), with modulate drains + square-accumulates folded per 512-col
chunk.  The conv-start critical chain is minimized: per-tap unit-stride
demod split across DVE/ACT with the demod-only taps (p=0,3) first,
per-kh unit-stride G-adds split DVE/Pool, chunk 1 runs p-order
(0,3,1,2), and an early dummy Sqrt keeps the ACT table resident.
Sample-1's ct1 mix runs on the PE as three small-PSUM passes between
sample-0 conv chunks; its x-transform and filter chain run as ~0.7us
DVE work items consumed between chunk drains.
"""

import os
import sys

import numpy as np

if "/opt/trn_rl_repo" not in sys.path:
    sys.path.insert(0, "/opt/trn_rl_repo")

import concourse.bacc as bacc_mod
import concourse.mybir as mybir
import concourse.tile as tile
from concourse.bass_utils import run_bass_kernel_spmd

N_CORES = 8
NS = 2            # samples per core
C = 256
F = 256
H = W = 64
KK = 3
TAPS = KK * KK    # 9
NF = 8
WD = 512
P = 128
CT = C // P       # 2 c tiles
FT = F // P       # 2 f tiles
KO = WD // P      # 4 contraction tiles for the dense layers
HP = H + 2        # 66 rows in the winograd input (h = -1..64)
NT = W // 2       # 32 output pairs per row
EO = NT + 2       # 34 cols per deinterleaved odd/even plane
HCH = 16          # output rows per conv chunk
CHN = H // HCH    # 4 chunks per (sample, f-tile)
CHL = HCH * NT    # 512 positions per chunk
MIX_CHUNKS = [(0, 512), (512, 512), (1024, 512), (1536, 512), (2048, 256)]
# taps covered by each mix chunk (tap t = cols [256t, 256t+256))
CHUNK_TAPS = [(0, 1), (2, 3), (4, 5), (6, 7), (8,)]
EPS = 1e-8

USE_BF16 = os.environ.get("KERNEL_F32", "") != "1"
WARM_MM = int(os.environ.get("KERNEL_WARM_MM", "2"))

LAST = None       # BassKernelResults of the most recent run (for test.py)


def _build():
    f32 = mybir.dt.float32
    cdt = mybir.dt.bfloat16 if USE_BF16 else f32

    nc = bacc_mod.Bacc()
    xdi_d = nc.declare_dram_parameter("xdi", [NS, CT, P, 2, HP, EO], cdt,
                                      isOutput=False)
    w_d = nc.declare_dram_parameter("wv_t", [P, KO, NS], f32, isOutput=False)
    bank_d = nc.declare_dram_parameter("bank_t", [NF, C, TAPS, F], cdt,
                                       isOutput=False)
    fww_d = nc.declare_dram_parameter("fw_w", [P, KO, NF], cdt, isOutput=False)
    fwb_d = nc.declare_dram_parameter("fw_b", [NF], cdt, isOutput=False)
    mdw_d = nc.declare_dram_parameter("md_w", [P, KO, F], cdt, isOutput=False)
    mdb_d = nc.declare_dram_parameter("md_b", [F], cdt, isOutput=False)
    ident_d = nc.declare_dram_parameter("ident", [P, P], cdt, isOutput=False)
    out_d = nc.declare_dram_parameter("out", [NS, F, H, W], f32, isOutput=True)

    mm = mybir.AluOpType.mult
    aa = mybir.AluOpType.add
    ss = mybir.AluOpType.subtract
    ACT = mybir.ActivationFunctionType

    with tile.TileContext(nc) as tc, \
         tc.tile_pool(name="const", bufs=1) as const_p, \
         tc.tile_pool(name="small", bufs=2) as small_p, \
         tc.tile_pool(name="bcast", bufs=2) as bc_p, \
         tc.tile_pool(name="diag", bufs=16) as diag_p, \
         tc.tile_pool(name="bank", bufs=16) as bk_p, \
         tc.tile_pool(name="xdi", bufs=2) as xdi_p, \
         tc.tile_pool(name="xt", bufs=8) as xt_p, \
         tc.tile_pool(name="acc", bufs=3) as acc_p, \
         tc.tile_pool(name="tmp", bufs=2) as tmp_p, \
         tc.tile_pool(name="filt", bufs=2) as filt_p, \
         tc.tile_pool(name="f12", bufs=2) as f12_p, \
         tc.tile_pool(name="scr", bufs=2) as scr_p, \
         tc.tile_pool(name="fs", bufs=2) as fs_p, \
         tc.tile_pool(name="outs", bufs=2) as out_p, \
         tc.tile_pool(name="ps", bufs=8, space="PSUM") as ps_p:

        # preload ACT tables (Exp/Square/Sqrt) with a self-contained tile
        tblw = small_p.tile([P, 2], f32, name="tblw")
        nc.vector.memset(tblw, 0.5)
        nc.scalar.activation(tblw, tblw, ACT.Exp)
        nc.scalar.activation(tblw, tblw, ACT.Square)
        nc.scalar.activation(tblw, tblw, ACT.Sqrt)

        # ---- bank DMA first (the long pole) on the sync ring; the ring
        # processes descriptors in order, so xq1 enqueued right after the
        # bank never steals bank bandwidth.
        bkr = [[None] * NF for _ in range(CT)]
        for ct in range(CT):
            for b in range(NF):
                bk = bk_p.tile([P, TAPS, F], cdt, tag="bk", name=f"bk{ct}_{b}")
                # two DMAs per tile, split on the tap-4 boundary (= mix
                # chunks 0-1 vs 2-4): overlap tracking then releases each
                # tile's first mix matmuls ~1.2us before the full tile
                # lands — including the b7 tile that gates conv start
                cs = ct * P
                # the first tile (starts the mix) and the last (gates conv
                # start) split per-chunk; the rest split 2-way — the
                # 16/32/80-descriptor sweep showed per-descriptor overhead
                # dominates beyond ~32
                if (ct, b) in ((0, 0), (1, NF - 1)):
                    pieces = ((0, 2), (2, 4), (4, 6), (6, 8), (8, 9))
                else:
                    pieces = ((0, 4), (4, 9))
                for t0, t1 in pieces:
                    nc.sync.dma_start(bk[:, t0:t1, :],
                                      bank_d[b, cs:cs + P, t0:t1, :])
                bkr[ct][b] = bk

        # ---- x sample 0 (gpsimd ring, races the bank) -------------------
        xq0 = []
        for ct in range(CT):
            xq = xdi_p.tile([P, 2, HP, EO], cdt, tag="xdi", name=f"xq0_{ct}")
            nc.gpsimd.dma_start(xq, xdi_d[0, ct, :, :, :, :])
            xq0.append(xq)

        # ---- x sample 1, behind the bank on the same sync ring: the ring
        # serves it only after the bank, and its descriptor wait (xdi buf
        # reuse, freed by the xt0 transforms) resolves before the ring
        # reaches it.
        xq1 = []
        for ct in range(CT):
            xq = xdi_p.tile([P, 2, HP, EO], cdt, tag="xdi", name=f"xq1_{ct}")
            nc.sync.dma_start(xq, xdi_d[1, ct, :, :, :, :])
            xq1.append(xq)

        # ---- small parameter loads (scalar ring) ------------------------
        # biases as single-partition rows, added via a K=1 matmul — the
        # [P,*] broadcast DMAs they replace were ring-hogs.
        wt = const_p.tile([P, KO, NS], f32)
        nc.scalar.dma_start(wt, w_d[:, :, :])
        fww = const_p.tile([P, KO, NF], cdt)
        nc.scalar.dma_start(fww, fww_d[:, :, :])
        fwb1 = const_p.tile([1, NF], cdt)
        nc.scalar.dma_start(fwb1, fwb_d[:][None, :])
        ident = const_p.tile([P, P], cdt)
        nc.scalar.dma_start(ident, ident_d[:, :])
        mdb1 = const_p.tile([1, F], cdt)
        nc.scalar.dma_start(mdb1, mdb_d[:][None, :])
        # mdw (256 KB, 85% of param bytes) rides the gpsimd ring BEHIND
        # xq0: its consumer chain (dense head -> w1rep -> mix modulate
        # drains) first needs it ~5us after it lands, and the scalar ring
        # finishing early gives the bank a bigger early bandwidth share.
        mdw = const_p.tile([P, KO, F], cdt)
        nc.gpsimd.dma_start(mdw, mdw_d[:, :, :])

        ones_row = const_p.tile([1, P], cdt)
        nc.vector.memset(ones_row, 1.0)

        # ---- dense heads, replicated across all 128 partitions ----------
        fbw_bc = []
        w1rep = []
        for n in range(NS):
            wrep = bc_p.tile([P, KO, P], cdt, tag="wrep")
            nc.vector.tensor_copy(
                wrep, wt[:, :, n][:, :, None].to_broadcast((P, KO, P)))
            ps_l = ps_p.tile([P, 512], f32, tag="ps", name=f"psl{n}")
            for ko in range(KO):
                nc.tensor.matmul(ps_l[:, :NF], lhsT=wrep[:, ko, :],
                                 rhs=fww[:, ko, :], start=(ko == 0),
                                 stop=False)
            nc.tensor.matmul(ps_l[:, :NF], lhsT=ones_row, rhs=fwb1,
                             start=False, stop=True)
            # softmax denominator cancels through the per-tap demod norm
            fb = bc_p.tile([P, NF], f32, tag="fbw_bc")
            nc.scalar.activation(fb, ps_l[:, :NF], ACT.Exp)
            fbw_bc.append(fb)

            ps_sc = ps_p.tile([P, 512], f32, tag="ps", name=f"pssc{n}")
            for ko in range(KO):
                nc.tensor.matmul(ps_sc[:, :F], lhsT=wrep[:, ko, :],
                                 rhs=mdw[:, ko, :], start=(ko == 0),
                                 stop=False)
            nc.tensor.matmul(ps_sc[:, :F], lhsT=ones_row, rhs=mdb1,
                             start=False, stop=True)
            w1 = bc_p.tile([P, F], cdt, tag="w1_bc")
            nc.scalar.activation(w1, ps_sc[:, :F], ACT.Copy, bias=1.0)
            w1r = bc_p.tile([P, TAPS, F], cdt, tag="w1rep")
            nc.vector.tensor_copy(
                w1r, w1[:, None, :].to_broadcast((P, TAPS, F)))
            w1rep.append(w1r)

        # warm psum: fresh short-lived tile (never held across pool phases)
        ps_wa = ps_p.tile([P, 512], f32, tag="ps", name="warm_a")
        ones_sb = const_p.tile([P, P], cdt)
        nc.vector.memset(ones_sb, 1.0)

        # diag(fbw_b) weight tiles for the PE mixes (samples 0 and 1)
        fbwI = [[], []]
        for n in range(NS):
            for b in range(NF):
                dg = diag_p.tile([P, P], cdt, tag="diag", name=f"dg{n}_{b}")
                nc.vector.tensor_scalar_mul(dg, ident, fbw_bc[n][:, b:b + 1])
                fbwI[n].append(dg)
                if WARM_MM and n == 0:
                    for _ in range(2):
                        nc.tensor.matmul(ps_wa[:, :P], lhsT=dg, rhs=dg,
                                         start=True, stop=True)

        # ---- x-tilde transform for sample 0 (DVE, unit-stride bf16) -----
        # xq[ct] planes: [:,0]=odd (w=2t-1), [:,1]=even (w=2t)
        def xt_transform(xq, pool, tag):
            xts = []
            for ct in range(CT):
                o = xq[ct][:, 0, :, :]
                e = xq[ct][:, 1, :, :]
                pl = []
                for p in range(4):
                    t = pool.tile([P, HP, NT], cdt, tag=tag,
                                  name=f"xt{tag}{ct}_{p}")
                    pl.append(t)
                nc.vector.tensor_tensor(pl[0], o[:, :, 0:NT], o[:, :, 1:NT + 1], ss)
                nc.vector.tensor_tensor(pl[1], e[:, :, 0:NT], o[:, :, 1:NT + 1], aa)
                nc.vector.tensor_tensor(pl[2], o[:, :, 1:NT + 1], e[:, :, 0:NT], ss)
                nc.vector.tensor_tensor(pl[3], e[:, :, 0:NT], e[:, :, 1:NT + 1], ss)
                xts.append(pl)
            return xts

        xt0 = xt_transform(xq0, xt_p, "xt")

        # ---- mix bookkeeping --------------------------------------------
        acc = [[None] * CT for _ in range(NS)]
        qt = [[None] * CT for _ in range(NS)]

        def mix_alloc(n, ct):
            a = acc_p.tile([P, TAPS, F], cdt, tag="acc", name=f"acc{n}_{ct}")
            scr = tmp_p.tile([P, TAPS, F], cdt, tag="tmp", name=f"sq{n}_{ct}")
            q = small_p.tile([P, TAPS], f32, tag="q", bufs=4, name=f"q{n}_{ct}")
            acc[n][ct] = a
            qt[n][ct] = q
            return a, scr, q

        a00, scr00, q00 = mix_alloc(0, 0)
        a10, scr10, q10 = mix_alloc(1, 0)
        a01, scr01, q01 = mix_alloc(0, 1)

        def mix_psums(n, ct, cis):
            return {ci: ps_p.tile([P, 512], f32, tag="ps",
                                  name=f"mix{n}{ct}_{ci}")
                    for ci in cis}

        def mix_mms(n, ct, pss, cis, b):
            bf = bkr[ct][b].rearrange("p t f -> p (t f)")
            for ci in cis:
                off, csz = MIX_CHUNKS[ci]
                nc.tensor.matmul(pss[ci][:, :csz], lhsT=fbwI[n][b],
                                 rhs=bf[:, off:off + csz],
                                 start=(b == 0), stop=(b == NF - 1))

        def mix_drain(n, ct, pss, cis, scr, q, eng):
            a = acc[n][ct]
            af = a.rearrange("p t f -> p (t f)")
            w1f = w1rep[n].rearrange("p t f -> p (t f)")
            for ci in cis:
                off, csz = MIX_CHUNKS[ci]
                eng.tensor_tensor(af[:, off:off + csz], pss[ci][:, :csz],
                                  w1f[:, off:off + csz], mm)
                for tp in CHUNK_TAPS[ci]:
                    nc.scalar.activation(scr[:, tp, :], a[:, tp, :],
                                         ACT.Square, accum_out=q[:, tp:tp + 1])

        # PHASE 1 (ct0 tiles): (0,0) full + (1,0) chunks 0-2; 8 psums live.
        ps00 = mix_psums(0, 0, range(5))
        ps10a = mix_psums(1, 0, range(3))
        for b in range(NF):
            mix_mms(0, 0, ps00, range(5), b)
            mix_mms(1, 0, ps10a, range(3), b)
        # drains must run on DVE/ACT (Pool cannot read PSUM)
        mix_drain(0, 0, ps00, range(5), scr00, q00, nc.vector)
        nc.scalar.activation(tblw, tblw, ACT.Sqrt)   # keep Sqrt table hot
        mix_drain(1, 0, ps10a, range(3), scr10, q10, nc.vector)

        # PHASE 2 (ct1 tiles): (0,1) full + (1,0) chunks 3-4; 7 psums live.
        ps01 = mix_psums(0, 1, range(5))
        ps10b = mix_psums(1, 0, (3, 4))
        for b in range(NF):
            mix_mms(0, 1, ps01, range(5), b)
            mix_mms(1, 0, ps10b, (3, 4), b)
        mix_drain(0, 1, ps01, range(5), scr01, q01, nc.vector)

        # ---- norm + demod + winograd filter transform for sample 0 ------
        def norm_chain(n):
            qs = small_p.tile([P, TAPS], cdt, tag="qs", name=f"qs{n}")
            nc.vector.tensor_tensor(qs, qt[n][0], qt[n][1], aa)
            ps_nrm = ps_p.tile([P, 512], f32, tag="ps", name=f"psn{n}")
            nc.tensor.matmul(ps_nrm[:, :TAPS], lhsT=ones_sb, rhs=qs,
                             start=True, stop=True)
            nall = small_p.tile([P, TAPS], f32, tag="nall", name=f"na{n}")
            nc.vector.tensor_scalar_max(nall, ps_nrm[:, :TAPS], EPS)
            sq = small_p.tile([P, TAPS], f32, tag="sq", name=f"sv{n}")
            nc.scalar.activation(sq, nall, ACT.Sqrt)
            ninv = small_p.tile([P, TAPS], f32, tag="ninv", name=f"ni{n}")
            nc.vector.reciprocal(ninv, sq)
            return ninv

        ninv0 = norm_chain(0)

        # filt holds the demodded taps (ft0 = tap kw0, ft3 = tap kw2);
        # f12 holds ft1 = g0+g1+g2 and ft2 = g0-g1+g2 per kh.  Demod runs
        # per-tap (unit-stride, per-partition 1/norm scalar) split across
        # DVE and ACT, taps for the demod-only conv planes (p=0,3) first so
        # chunk 1 can start; the G-adds run per-kh unit-stride, ct0 on DVE
        # and ct1 on Pool.
        DEMOD_ORDER = (0, 3, 6, 2, 5, 8, 1, 4, 7)

        def filter_transform(n, ninv, fl_tiles, fx_tiles):
            for i, tp in enumerate(DEMOD_ORDER):
                for ct in range(CT):
                    src = acc[n][ct][:, tp, :]
                    dst = fl_tiles[ct][:, tp, :]
                    if (2 * i + ct) % 2 == 0:
                        nc.vector.tensor_scalar_mul(dst, src,
                                                    ninv[:, tp:tp + 1])
                    else:
                        nc.scalar.activation(dst, src, ACT.Copy,
                                             scale=ninv[:, tp:tp + 1])
            for ct in range(CT):
                fl, fx = fl_tiles[ct], fx_tiles[ct]
                eng = nc.vector if ct == 0 else nc.gpsimd
                s3 = fs_p.tile([P, KK, F], cdt, tag="fs", name=f"fs{n}_{ct}")
                for kh in range(KK):
                    eng.tensor_tensor(s3[:, kh, :], fl[:, kh * KK, :],
                                      fl[:, kh * KK + 2, :], aa)
                    eng.tensor_tensor(fx[:, 0, kh, :], s3[:, kh, :],
                                      fl[:, kh * KK + 1, :], aa)
                    eng.tensor_tensor(fx[:, 1, kh, :], s3[:, kh, :],
                                      fl[:, kh * KK + 1, :], ss)

        filt0 = [filt_p.tile([P, TAPS, F], cdt, tag="filt", name=f"fl0_{ct}")
                 for ct in range(CT)]
        f120 = [f12_p.tile([P, 2, KK, F], cdt, tag="f12", name=f"fx0_{ct}")
                for ct in range(CT)]
        filter_transform(0, ninv0, filt0, f120)

        # sample-1 ct0 tail drains AFTER the s0 filter chain: they overlap
        # conv chunk 1 instead of sitting on the conv-start critical path.
        mix_drain(1, 0, ps10b, (3, 4), scr10, q10, nc.vector)

        acc11 = bk_p.tile([P, TAPS, F], cdt, tag="bk", name="acc1_1")

        # ---- conv + drains, interleaved with sample-1 prep --------------
        def conv_sample(n, filt, f12, xts, vwork, per_chunk, post_mm=None):
            """vwork: list of callables issuing ~0.8us of engine work each,
            consumed per_chunk at a time between chunk drains.  post_mm:
            {chunk_idx: fn} PE work inserted right after that chunk's MMs."""
            wi = 0
            first = True
            ch = 0
            for ft in range(FT):
                fsl = slice(ft * P, (ft + 1) * P)
                for hc in range(CHN):
                    pss = [ps_p.tile([P, CHL], f32, tag="ps",
                                     name=f"cv{n}{ft}{hc}_{p}")
                           for p in range(4)]
                    # first chunk: run the demod-only taps (p=0,3) first so
                    # the PE can start before the f12 G-adds are done.
                    p_order = (0, 3, 1, 2) if first else (1, 2, 0, 3)
                    first = False
                    for p in p_order:
                        k = 0
                        for kh in range(KK):
                            for ct in range(CT):
                                if p == 0:
                                    lhs = filt[ct][:, kh * KK, fsl]
                                elif p == 3:
                                    lhs = filt[ct][:, kh * KK + 2, fsl]
                                else:
                                    lhs = f12[ct][:, p - 1, kh, fsl]
                                xf = xts[ct][p].rearrange("p h t -> p (h t)")
                                off = (hc * HCH + kh) * NT
                                nc.tensor.matmul(
                                    pss[p][:, :], lhsT=lhs,
                                    rhs=xf[:, off:off + CHL],
                                    start=(k == 0), stop=(k == 2 * KK - 1))
                                k += 1
                    if post_mm and ch in post_mm:
                        post_mm[ch]()
                    ch += 1
                    osb = out_p.tile([P, HCH, W], f32, tag="osb",
                                     name=f"osb{n}{ft}{hc}")
                    ov = osb.rearrange("p h (t two) -> p h t two", two=2)
                    a1 = scr_p.tile([P, HCH, NT], f32, tag="a1",
                                    name=f"a1_{n}{ft}{hc}")
                    a2 = scr_p.tile([P, HCH, NT], f32, tag="a2",
                                    name=f"a2_{n}{ft}{hc}")
                    dv = scr_p.tile([P, HCH, NT], f32, tag="d",
                                    name=f"d{n}{ft}{hc}")
                    p1 = pss[1].rearrange("p (h t) -> p h t", t=NT)
                    p2 = pss[2].rearrange("p (h t) -> p h t", t=NT)
                    p0 = pss[0].rearrange("p (h t) -> p h t", t=NT)
                    p3 = pss[3].rearrange("p (h t) -> p h t", t=NT)
                    # one-PSUM-input-per-op drain, spread ACT/Pool/DVE:
                    #   a1 = 0.5*y1 ; a2 = 0.5*y2 (ACT, psum->sbuf)
                    #   d = a1 - a2 (Pool) ; a1 <- a1 + a2 (DVE)
                    #   even = (a1+a2) + y0 ; odd = -y3 + d (DVE)
                    # the very last chunk drains in row-halves so the final
                    # DMA (and the epilogue behind it) starts sooner
                    last = (n == 1 and ft == FT - 1 and hc == CHN - 1)
                    for r0, r1 in ([(0, 8), (8, HCH)] if last
                                   else [(0, HCH)]):
                        rs = slice(r0, r1)
                        nc.scalar.activation(a1[:, rs], p1[:, rs],
                                             ACT.Copy, scale=0.5)
                        nc.scalar.activation(a2[:, rs], p2[:, rs],
                                             ACT.Copy, scale=0.5)
                        nc.gpsimd.tensor_tensor(dv[:, rs], a1[:, rs],
                                                a2[:, rs], ss)
                        nc.vector.tensor_tensor(a1[:, rs], a1[:, rs],
                                                a2[:, rs], aa)
                        nc.vector.tensor_tensor(ov[:, rs, :, 0], a1[:, rs],
                                                p0[:, rs], aa)
                        nc.vector.scalar_tensor_tensor(ov[:, rs, :, 1],
                                                       p3[:, rs], -1.0,
                                                       dv[:, rs], mm, aa)
                        nc.sync.dma_start(
                            out_d[n, fsl, hc * HCH + r0:hc * HCH + r1, :],
                            osb[:, rs])
                    for _ in range(per_chunk):
                        if wi < len(vwork):
                            vwork[wi]()
                            wi += 1
            while wi < len(vwork):
                vwork[wi]()
                wi += 1

        # ---- sample-1 ct1 mix on the PE, interleaved between s0 conv
        # chunks in two PSUM-budgeted passes (p1 = chunks 0-2, p2 = 3-4).
        scr11 = tmp_p.tile([P, TAPS, F], cdt, tag="tmp", name="sq1_1")
        q11 = small_p.tile([P, TAPS], f32, tag="q", bufs=4, name="q1_1")
        acc[1][1] = acc11
        qt[1][1] = q11

        def mk_mix11_pass(cis):
            # one full 8-bank accumulation over <=2 psum chunks + drain;
            # inserted between s0 conv chunks (live psums: conv 4 + 2)
            def fn():
                pss = mix_psums(1, 1, cis)
                # high_priority: schedule the pass as one tight PE block —
                # left alone the scheduler interleaves these 1:1 with conv
                # matmuls, doubling the time until the drain can run and
                # head-of-line-blocking the DVE queue behind it
                with tc.high_priority(offset=60):
                    for b in range(NF):
                        mix_mms(1, 1, pss, cis, b)
                mix_drain(1, 1, pss, cis, scr11, q11, nc.vector)
            return fn

        mix11_passes = {1: mk_mix11_pass((0, 1)), 3: mk_mix11_pass((2, 3)),
                        5: mk_mix11_pass((4,))}

        filt1 = [bk_p.tile([P, TAPS, F], cdt, tag="bk", name=f"fl1_{ct}")
                 for ct in range(CT)]
        f121 = [bk_p.tile([P, 2, KK, F], cdt, tag="bk", name=f"fx1_{ct}")
                for ct in range(CT)]

        def mk_norm_filt1():
            def fn():
                ninv1 = norm_chain(1)
                filter_transform(1, ninv1, filt1, f121)
            return fn

        def mk_xt1(ct, p, half, eng):
            # half-plane slices of the sample-1 x-tilde transform
            r0, r1 = (0, HP // 2 + 1) if half == 0 else (HP // 2 + 1, HP)
            def fn():
                o = xq1[ct][:, 0, r0:r1, :]
                e = xq1[ct][:, 1, r0:r1, :]
                t = bk_p.tile([P, HP, NT], cdt, tag="bk", name=f"xt1{ct}_{p}") \
                    if half == 0 else xt1[ct][p]
                d = t[:, r0:r1, :]
                if p == 0:
                    eng.tensor_tensor(d, o[:, :, 0:NT], o[:, :, 1:NT + 1], ss)
                elif p == 1:
                    eng.tensor_tensor(d, e[:, :, 0:NT], o[:, :, 1:NT + 1], aa)
                elif p == 2:
                    eng.tensor_tensor(d, o[:, :, 1:NT + 1], e[:, :, 0:NT], ss)
                else:
                    eng.tensor_tensor(d, e[:, :, 0:NT], e[:, :, 1:NT + 1], ss)
                if half == 0:
                    xt1[ct][p] = t
            return fn

        xt1 = [[None] * 4 for _ in range(CT)]
        xt1_items = [mk_xt1(ct, p, half, nc.vector)
                     for ct in range(CT) for p in range(4)
                     for half in range(2)]
        # norm1 placed so its PE matmul fires ~1 chunk after the last
        # mix(1,1) squares land (q11 ready ~chunk 5.5)
        vwork = xt1_items[:15] + [mk_norm_filt1()] + xt1_items[15:]

        conv_sample(0, filt0, f120, xt0, vwork, per_chunk=3,
                    post_mm=mix11_passes)
        conv_sample(1, filt1, f121, xt1, [], per_chunk=0)

    nc.compile()
    return nc


def kernel(x, w, filter_bank, dense_fw_w, dense_fw_b, dense_mod_w, dense_mod_b):
    global LAST
    x = np.ascontiguousarray(np.asarray(x, dtype=np.float32))
    w = np.ascontiguousarray(np.asarray(w, dtype=np.float32))
    xdt = np.float32
    if USE_BF16:
        import ml_dtypes
        xdt = ml_dtypes.bfloat16
    NB = x.shape[0]
    # deinterleaved, padded odd/even planes:
    #   odd[t]  = x[w=2t-1] (t=0..33, zeros at w=-1 and w=65)
    #   even[t] = x[w=2t]   (t=0..33, zeros at w=64.. )
    # rows r=0..65 map to h=r-1 with zero padding at h=-1, 64.
    xr = x.reshape(NB, CT, P, H, W)
    xdi_all = np.zeros((NB, CT, P, 2, HP, EO), dtype=xdt)
    xdi_all[:, :, :, 0, 1:H + 1, 1:NT + 1] = xr[:, :, :, :, 1::2]
    xdi_all[:, :, :, 1, 1:H + 1, 0:NT] = xr[:, :, :, :, 0::2]
    fb = np.asarray(filter_bank, dtype=np.float32)
    # [b, f, c, kh, kw] -> [b, c, (kh kw), f]
    bank_t = np.ascontiguousarray(
        np.transpose(fb, (0, 2, 3, 4, 1)).reshape(NF, C, TAPS, F))
    if USE_BF16:
        import ml_dtypes
        bank_t = bank_t.astype(ml_dtypes.bfloat16)

    trace = os.environ.get("KERNEL_TRACE", "") == "1"
    if trace:
        import types

        import concourse.bass_utils as bu
        bu.upload_artifacts = lambda tmpdir: tmpdir
        if "antenv.axon_hooks" not in sys.modules:
            from trn_agent_boot.trn_boot import _ntff_profile_via_ctypes
            hook = _ntff_profile_via_ctypes("/opt/axon/libaxon_pjrt.so")
            mod = types.ModuleType("antenv.axon_hooks")
            mod.get_axon_ntff_profile_hook = lambda: hook
            sys.modules["antenv.axon_hooks"] = mod

    # dense weights pre-transposed to [P, KO, *] so the DMA is contiguous
    fww_t = np.ascontiguousarray(
        np.asarray(dense_fw_w, np.float32).reshape(KO, P, NF)
        .transpose(1, 0, 2).astype(xdt))
    mdw_t = np.ascontiguousarray(
        np.asarray(dense_mod_w, np.float32).reshape(KO, P, F)
        .transpose(1, 0, 2).astype(xdt))

    ncores = int(os.environ.get("KERNEL_NCORES", N_CORES))
    nc = _build()
    in_maps = []
    for core in range(ncores):
        sl = slice(core * NS, (core + 1) * NS)
        w_t = np.ascontiguousarray(
            w[sl].reshape(NS, KO, P).transpose(2, 1, 0))
        ident = np.eye(P, dtype=xdt)
        in_maps.append({
            "ident": ident,
            "xdi": np.ascontiguousarray(xdi_all[sl]),
            "wv_t": w_t,
            "bank_t": bank_t,
            "fw_w": fww_t,
            "fw_b": np.ascontiguousarray(
                np.asarray(dense_fw_b, np.float32).astype(xdt)),
            "md_w": mdw_t,
            "md_b": np.ascontiguousarray(
                np.asarray(dense_mod_b, np.float32).astype(xdt)),
        })
    kwargs = {}
    if trace:
        import tempfile
        base = os.environ.get("KERNEL_TRACE_DIR", "/tmp/ktrace")
        os.makedirs(base, exist_ok=True)
        tdir = tempfile.mkdtemp(dir=base)
        print(f"trace dir: {tdir}", flush=True)
        kwargs = dict(trace=True, tmpdir=tdir)
    LAST = run_bass_kernel_spmd(nc, in_maps, core_ids=list(range(ncores)),
                                **kwargs)
    return np.concatenate([LAST.results[i]["out"] for i in range(ncores)],
                          axis=0)
